# revision 27
# baseline (speedup 1.0000x reference)
"""Trainium2 Bass kernel for nn_MlroleNode_64716567216639 (GAT message passing).

Math note: the reference model computes a dense NxN GATv2 attention but only
row 0 of the output (gat_out[0]) feeds the final MLP, so this kernel computes
just that row: e[j,h] = leaky(g_l[j] + g_r[0]) . w_attn, softmax over the 1024
source nodes, then a weighted sum of g_r values, followed by the 3-layer
type-define MLP over the 1023 ambiguous nodes.

Layout: features on partitions, nodes on the free axis (everything transposed
on host). The GAT row-0 computation is replicated on all 8 cores; the final
MLP is sharded 128 nodes per core.
"""
import numpy as np

H = 64
N_AMB = 1023
N = 1024
HEADS = 4
HID = 64
RT = 4
APT = 3
SLOPE = 0.2
NCORES = 8
SHARD = 128  # MLP nodes per core (8*128 = 1024 = N_AMB padded by 1)

_compiled = None  # (nc, static_in_maps_builder)


def _build():
    import concourse.tile as tile
    from concourse import bacc, mybir

    dt = mybir.dt.float32
    AF = mybir.ActivationFunctionType
    ALU = mybir.AluOpType
    AX = mybir.AxisListType

    nc = bacc.Bacc("TRN2", target_bir_lowering=False, debug=False,
                   enable_asserts=False, num_devices=NCORES)

    def din(name, shape):
        return nc.dram_tensor(name, shape, dt, kind="ExternalInput").ap()

    ambT_d = din("ambT", [H, N_AMB])
    hidc_d = din("hidc", [H, 1])
    ta_d = din("ta", [H, RT * APT])
    WselfT_d = din("WselfT", [H, H])
    WmLT_d = din("WmLT", [H, H])
    WmRT_d = din("WmRT", [H, H])
    WtT_d = din("WtT", [H, RT * H])
    btT_d = din("btT", [H, RT])
    bsc_d = din("bsc", [H, 1])
    bmc_d = din("bmc", [H, 1])
    WlT0_d = din("WlT0", [H, 128])
    WlT1_d = din("WlT1", [H, 128])
    WrT_d = din("WrT", [H, HEADS * HID])
    Wexp_d = din("Wexp", [128, 128])
    fold_d = din("fold", [128, H])
    Wd0a_aug_d = din("Wd0a_aug", [H + 1, 64])
    Wd0bT_d = din("Wd0bT", [H, 64])
    Wd1_aug_d = din("Wd1_aug", [65, 128])
    Wd2T_d = din("Wd2T", [128, RT])
    bd2c_d = din("bd2c", [RT, 1])
    mlp_d = din("mlp_cols", [H, SHARD])
    outT_d = nc.dram_tensor("outT", [RT, SHARD], dt, kind="ExternalOutput").ap()

    with tile.TileContext(nc) as tc:
        with tc.tile_pool(name="wp", bufs=1) as wp, \
             tc.tile_pool(name="sb", bufs=1) as sb, \
             tc.tile_pool(name="ps", bufs=1, space="PSUM") as ps:

            # ---- load inputs to SBUF ----
            def load(dram_ap, shape, tag):
                t = wp.tile(shape, dt, tag=tag)
                nc.sync.dma_start(t[:], dram_ap[:])
                return t

            ta_sb = load(ta_d, [H, RT * APT], "ta")
            WselfT = load(WselfT_d, [H, H], "WselfT")
            WmLT = load(WmLT_d, [H, H], "WmLT")
            WmRT = load(WmRT_d, [H, H], "WmRT")
            WtT = load(WtT_d, [H, RT * H], "WtT")
            btT = load(btT_d, [H, RT], "btT")
            bsc = load(bsc_d, [H, 1], "bsc")
            bmc = load(bmc_d, [H, 1], "bmc")
            hidc = load(hidc_d, [H, 1], "hidc")
            WlT = [load(WlT0_d, [H, 128], "WlT0"), load(WlT1_d, [H, 128], "WlT1")]
            WrT = load(WrT_d, [H, HEADS * HID], "WrT")
            Wexp = load(Wexp_d, [128, 128], "Wexp")
            fold = load(fold_d, [128, H], "fold")
            Wd0a_aug = load(Wd0a_aug_d, [H + 1, 64], "Wd0a")
            Wd0bT = load(Wd0bT_d, [H, 64], "Wd0b")
            Wd1_aug = load(Wd1_aug_d, [65, 128], "Wd1")
            Wd2T = load(Wd2T_d, [128, RT], "Wd2")
            bd2c = load(bd2c_d, [RT, 1], "bd2c")

            hT = wp.tile([H, N], dt, tag="hT")
            nc.sync.dma_start(hT[:, 1:N], ambT_d[:])
            mlp_aug = wp.tile([H + 1, SHARD], dt, tag="mlpa")
            nc.sync.dma_start(mlp_aug[0:H, :], mlp_d[:])
            nc.vector.memset(mlp_aug[H:H + 1, :], 1.0)
            # preload ACT tables (Exp/Sigmoid) off the critical softmax path
            warm = wp.tile([1, 4], dt, tag="warm")
            nc.vector.memset(warm[:], 0.0)
            warm_act = wp.tile([1, 4], dt, tag="warmact")
            nc.scalar.activation(warm_act[0:1, 0:1], warm[0:1, 0:1], AF.Exp)

            def leaky(out_ap, in_ap):
                # in_ap must be SBUF (stt can read at most one PSUM input)
                nc.vector.scalar_tensor_tensor(out=out_ap, in0=in_ap, scalar=SLOPE,
                                               in1=in_ap, op0=ALU.mult, op1=ALU.max)

            def leaky_psum(out_ap, psum_ap, scratch_ap):
                # leaky(x) = max(0.2*x, x) with x in PSUM: two DVE ops
                nc.vector.tensor_scalar_mul(scratch_ap, psum_ap, SLOPE)
                nc.vector.tensor_tensor(out_ap, scratch_ap, psum_ap, op=ALU.max)

            # ---- prologue: role-type routing + merge chain -> h1 [64,1] ----
            tsum = sb.tile([H, RT], dt, tag="tsum")
            nc.vector.reduce_sum(tsum[:], ta_sb[:].rearrange("p (t a) -> p t a", a=APT),
                                 axis=AX.X)
            tmean = sb.tile([H, RT], dt, tag="tmean")
            nc.vector.tensor_scalar_mul(tmean[:], tsum[:], 1.0 / APT)
            tmp_ps = ps.tile([H, RT], dt, tag="sp", bufs=1)
            for t in range(RT):
                nc.tensor.matmul(tmp_ps[:, t:t + 1], WtT[:, H * t:H * (t + 1)],
                                 tmean[:, t:t + 1], start=True, stop=True)
            tmpc = sb.tile([H, RT], dt, tag="tmpc")
            nc.vector.tensor_add(tmpc[:], tmp_ps[:], btT[:])
            C_ps = ps.tile([H, RT], dt, tag="sp", bufs=1)
            nc.tensor.matmul(C_ps[:], WmRT[:], tmpc[:], start=True, stop=True)
            C_sb = sb.tile([H, RT], dt, tag="C")
            nc.scalar.activation(C_sb[:], C_ps[:], AF.Identity, bias=bmc[:])

            h1_ps = ps.tile([H, 1], dt, tag="sp", bufs=1)
            nc.tensor.matmul(h1_ps[:], WselfT[:], hidc[:], start=True, stop=True)
            h1 = sb.tile([H, 1], dt, tag="h1", bufs=2)
            nc.scalar.activation(h1[:], h1_ps[:], AF.Identity, bias=bsc[:])
            for t in range(RT):
                hp = ps.tile([H, 1], dt, tag="sp", bufs=1)
                nc.tensor.matmul(hp[:], WmLT[:], h1[:], start=True, stop=True)
                u = sb.tile([H, 1], dt, tag="u", bufs=2)
                nc.scalar.activation(u[:], hp[:], AF.Identity, bias=C_sb[:, t:t + 1])
                h1n = sb.tile([H, 1], dt, tag="h1", bufs=2)
                leaky(h1n[:], u[:])
                h1 = h1n
            nc.vector.tensor_copy(hT[:, 0:1], h1[:])

            # ---- GAT row 0, two head-pair blocks ----
            h2_ps = ps.tile([H, 1], dt, tag="h2ps", bufs=1)
            for b in range(2):
                # g_r0 column for this head-pair block (attention query side)
                gr0_ps = ps.tile([128, 1], dt, tag="sp", bufs=1)
                nc.tensor.matmul(gr0_ps[:], WrT[:, 128 * b:128 * b + 128], h1[:],
                                 start=True, stop=True)
                gr0c = sb.tile([128, 1], dt, tag="gr0", bufs=2)
                nc.vector.tensor_copy(gr0c[:], gr0_ps[:])
                # cols 1.. depend only on the input DMA and can overlap the
                # serial prologue on the PE; col 0 (h1) is emitted last
                gl_ps = ps.tile([128, N], dt, tag="gle", bufs=2)
                nc.tensor.matmul(gl_ps[:, 1:512], WlT[b][:], hT[:, 1:512],
                                 start=True, stop=True)
                nc.tensor.matmul(gl_ps[:, 512:N], WlT[b][:], hT[:, 512:N],
                                 start=True, stop=True)
                nc.tensor.matmul(gl_ps[:, 0:1], WlT[b][:], hT[:, 0:1],
                                 start=True, stop=True)
                t_sb = sb.tile([128, N], dt, tag="t", bufs=2)
                for c in (0, 512):
                    u_sb = sb.tile([128, 512], dt, tag="scr", bufs=2)
                    nc.scalar.activation(u_sb[:], gl_ps[:, c:c + 512], AF.Identity,
                                         bias=gr0c[:])
                    leaky(t_sb[:, c:c + 512], u_sb[:])
                gr_ps = ps.tile([128, N], dt, tag="gr", bufs=1)
                nc.tensor.matmul(gr_ps[:, 1:512], WrT[:, 128 * b:128 * b + 128],
                                 hT[:, 1:512], start=True, stop=True)
                nc.tensor.matmul(gr_ps[:, 512:N], WrT[:, 128 * b:128 * b + 128],
                                 hT[:, 512:N], start=True, stop=True)
                nc.tensor.matmul(gr_ps[:, 0:1], WrT[:, 128 * b:128 * b + 128],
                                 hT[:, 0:1], start=True, stop=True)
                e_ps = ps.tile([128, N], dt, tag="gle", bufs=2)
                for c in (0, 512):
                    nc.tensor.matmul(e_ps[:, c:c + 512], Wexp[:], t_sb[:, c:c + 512],
                                     start=True, stop=True)
                # softmax over the 1024 source nodes (per head, replicated x64).
                # logits are O(5) so no max subtraction is needed in fp32.
                pexp = sb.tile([128, N], dt, tag="pexp", bufs=2)
                sa = sb.tile([128, 1], dt, tag="s", bufs=4)
                sbb = sb.tile([128, 1], dt, tag="s", bufs=4)
                nc.scalar.activation(pexp[:, 0:512], e_ps[:, 0:512], AF.Exp,
                                     bias=0.0, accum_out=sa[:])
                nc.scalar.activation(pexp[:, 512:N], e_ps[:, 512:N], AF.Exp,
                                     bias=0.0, accum_out=sbb[:])
                ssum = sb.tile([128, 1], dt, tag="s", bufs=4)
                nc.vector.tensor_add(ssum[:], sa[:], sbb[:])
                # weighted value sum over source nodes (fused mul + row-sum)
                acc = []
                for c in (0, 512):
                    scr = sb.tile([128, 512], dt, tag="scr", bufs=2)
                    a_c = sb.tile([128, 1], dt, tag="acc", bufs=4)
                    nc.vector.scalar_tensor_tensor(
                        out=scr[:], in0=pexp[:, c:c + 512], scalar=1.0,
                        in1=gr_ps[:, c:c + 512], op0=ALU.mult, op1=ALU.mult,
                        accum_out=a_c[:])
                    acc.append(a_c)
                att_u = sb.tile([128, 1], dt, tag="acc", bufs=4)
                nc.vector.tensor_add(att_u[:], acc[0][:], acc[1][:])
                rs = sb.tile([128, 1], dt, tag="s", bufs=4)
                nc.vector.reciprocal(rs[:], ssum[:])
                att_n = sb.tile([128, 1], dt, tag="acc", bufs=4)
                nc.vector.tensor_mul(att_n[:], att_u[:], rs[:])
                # fold heads: h2 += 0.25 * sum over the 2 heads in this block
                nc.tensor.matmul(h2_ps[:], fold[:], att_n[:], start=(b == 0),
                                 stop=(b == 1))

            h2 = sb.tile([H, 1], dt, tag="h2")
            nc.vector.tensor_copy(h2[:], h2_ps[:])

            # ---- final MLP on this core's 128-node shard ----
            c0_ps = ps.tile([H, 1], dt, tag="sp", bufs=1)
            nc.tensor.matmul(c0_ps[:], Wd0bT[:], h2[:], start=True, stop=True)
            c0col = sb.tile([H, 1], dt, tag="c0")
            nc.vector.tensor_copy(c0col[:], c0_ps[:])
            y0_ps = ps.tile([64, SHARD], dt, tag="sp", bufs=1)
            nc.tensor.matmul(y0_ps[:], Wd0a_aug[:], mlp_aug[:], start=True, stop=True)
            y0_aug = sb.tile([65, SHARD], dt, tag="y0")
            nc.vector.memset(y0_aug[64:65, :], 1.0)
            y0u = sb.tile([64, SHARD], dt, tag="yscr", bufs=2)
            nc.scalar.activation(y0u[:], y0_ps[:], AF.Identity, bias=c0col[:])
            leaky(y0_aug[0:64, :], y0u[:])
            y1_ps = ps.tile([128, SHARD], dt, tag="sp", bufs=1)
            nc.tensor.matmul(y1_ps[:], Wd1_aug[:], y0_aug[:], start=True, stop=True)
            y1 = sb.tile([128, SHARD], dt, tag="y1")
            y1scr = sb.tile([128, SHARD], dt, tag="yscr", bufs=2)
            leaky_psum(y1[:], y1_ps[:], y1scr[:])
            o_ps = ps.tile([RT, SHARD], dt, tag="sp", bufs=1)
            nc.tensor.matmul(o_ps[:], Wd2T[:], y1[:], start=True, stop=True)
            # sigmoid(z) = 1/(1+exp(-z)) using the already-loaded Exp table
            # (avoids a 1.3us Sigmoid ACT-table load on the critical path)
            o_e = sb.tile([RT, SHARD], dt, tag="oe")
            nc.scalar.activation(o_e[:], o_ps[:], AF.Exp, bias=bd2c[:], scale=-1.0)
            o_1p = sb.tile([RT, SHARD], dt, tag="o1p")
            nc.vector.tensor_scalar_add(o_1p[:], o_e[:], 1.0)
            o_sb = sb.tile([RT, SHARD], dt, tag="o")
            nc.vector.reciprocal(o_sb[:], o_1p[:])
            nc.sync.dma_start(outT_d[:], o_sb[:])

    nc.compile()
    return nc


def _prep_inputs(inputs):
    f32 = np.float32

    def c(a):
        return np.ascontiguousarray(a, dtype=f32)

    hidden = np.asarray(inputs["hidden"], f32)
    ambiguous = np.asarray(inputs["ambiguous"], f32)
    type_agents = np.asarray(inputs["type_agents"], f32)
    W_self = np.asarray(inputs["W_self"], f32)
    b_self = np.asarray(inputs["b_self"], f32)
    W_merge = np.asarray(inputs["W_merge"], f32)
    b_merge = np.asarray(inputs["b_merge"], f32)
    W_trans = np.asarray(inputs["W_trans"], f32)
    b_trans = np.asarray(inputs["b_trans"], f32)
    W_l = np.asarray(inputs["W_l"], f32)
    W_r = np.asarray(inputs["W_r"], f32)
    w_attn = np.asarray(inputs["w_attn"], f32)
    Wd0 = np.asarray(inputs["Wd0"], f32)
    bd0 = np.asarray(inputs["bd0"], f32)
    Wd1 = np.asarray(inputs["Wd1"], f32)
    bd1 = np.asarray(inputs["bd1"], f32)
    Wd2 = np.asarray(inputs["Wd2"], f32)
    bd2 = np.asarray(inputs["bd2"], f32)

    ambT = c(ambiguous.T)                                   # [64, 1023]
    WlT_full = c(W_l.T)                                     # [64, 256]
    Wexp = np.zeros((128, 128), f32)
    for hh in range(2):
        Wexp[hh * 64:(hh + 1) * 64, hh * 64:(hh + 1) * 64] = w_attn[:, None]
    fold = np.zeros((128, 64), f32)
    fold[np.arange(128), np.arange(128) % 64] = 0.25

    shared = {
        "ambT": ambT,
        "hidc": c(hidden.reshape(H, 1)),
        "ta": c(type_agents.reshape(RT * APT, H).T),
        "WselfT": c(W_self.T),
        "WmLT": c(W_merge[:, :H].T),
        "WmRT": c(W_merge[:, H:].T),
        "WtT": c(np.concatenate([W_trans[t].T for t in range(RT)], axis=1)),
        "btT": c(b_trans.T),
        "bsc": c(b_self.reshape(H, 1)),
        "bmc": c(b_merge.reshape(H, 1)),
        "WlT0": c(WlT_full[:, :128]),
        "WlT1": c(WlT_full[:, 128:]),
        "WrT": c(W_r.T),
        "Wexp": Wexp,
        "fold": fold,
        "Wd0a_aug": c(np.vstack([Wd0[:, :H].T, bd0[None, :]])),
        "Wd0bT": c(Wd0[:, H:].T),
        "Wd1_aug": c(np.vstack([Wd1.T, bd1[None, :]])),
        "Wd2T": c(Wd2.T),
        # negated: used as the bias of Exp(-z) inside the exp-based sigmoid
        "bd2c": c(-bd2.reshape(RT, 1)),
    }
    amb_pad = np.zeros((H, NCORES * SHARD), f32)
    amb_pad[:, :N_AMB] = ambT
    in_maps = []
    for cidx in range(NCORES):
        m = dict(shared)
        m["mlp_cols"] = c(amb_pad[:, cidx * SHARD:(cidx + 1) * SHARD])
        in_maps.append(m)
    return in_maps


def kernel(**inputs) -> np.ndarray:
    global _compiled
    if _compiled is None:
        _compiled = _build()
    nc = _compiled
    from concourse import bass_utils

    in_maps = _prep_inputs(inputs)
    res = bass_utils.run_bass_kernel_spmd(nc, in_maps, core_ids=list(range(NCORES)))
    out = np.empty((N_AMB, RT), np.float32)
    for cidx in range(NCORES):
        lo = cidx * SHARD
        hi = min(lo + SHARD, N_AMB)
        out[lo:hi, :] = res.results[cidx]["outT"][:, :hi - lo].T
    return out


# revision 28
# speedup vs baseline: 1.0451x; 1.0451x over previous
"""Trainium2 Bass kernel for nn_MlroleNode_64716567216639 (GAT message passing).

Math note: the reference model computes a dense NxN GATv2 attention but only
row 0 of the output (gat_out[0]) feeds the final MLP, so this kernel computes
just that row: e[j,h] = leaky(g_l[j] + g_r[0]) . w_attn, softmax over the 1024
source nodes, then a weighted sum of g_r values, followed by the 3-layer
type-define MLP over the 1023 ambiguous nodes.

Layout: features on partitions, nodes on the free axis (everything transposed
on host). The GAT row-0 computation is replicated on all 8 cores; the final
MLP is sharded 128 nodes per core.
"""
import numpy as np

H = 64
N_AMB = 1023
N = 1024
HEADS = 4
HID = 64
RT = 4
APT = 3
SLOPE = 0.2
NCORES = 8
SHARD = 128  # MLP nodes per core (8*128 = 1024 = N_AMB padded by 1)

_compiled = None  # (nc, static_in_maps_builder)


def _build():
    import concourse.tile as tile
    from concourse import bacc, mybir

    dt = mybir.dt.float32
    AF = mybir.ActivationFunctionType
    ALU = mybir.AluOpType
    AX = mybir.AxisListType

    nc = bacc.Bacc("TRN2", target_bir_lowering=False, debug=False,
                   enable_asserts=False, num_devices=NCORES)

    def din(name, shape):
        return nc.dram_tensor(name, shape, dt, kind="ExternalInput").ap()

    ambT_d = din("ambT", [H, N_AMB])
    hidc_d = din("hidc", [H, 1])
    ta_d = din("ta", [H, RT * APT])
    WselfT_d = din("WselfT", [H, H])
    WmLT_d = din("WmLT", [H, H])
    WmRT_d = din("WmRT", [H, H])
    WtT_d = din("WtT", [H, RT * H])
    btT_d = din("btT", [H, RT])
    bsc_d = din("bsc", [H, 1])
    bmc_d = din("bmc", [H, 1])
    WlT0_d = din("WlT0", [H, 128])
    WlT1_d = din("WlT1", [H, 128])
    WrT_d = din("WrT", [H, HEADS * HID])
    Wexp_d = din("Wexp", [128, 128])
    fold_d = din("fold", [128, H])
    Wd0a_aug_d = din("Wd0a_aug", [H + 1, 64])
    Wd0bT_d = din("Wd0bT", [H, 64])
    Wd1_aug_d = din("Wd1_aug", [65, 128])
    Wd2T_d = din("Wd2T", [128, RT])
    bd2c_d = din("bd2c", [RT, 1])
    mlp_d = din("mlp_cols", [H, SHARD])
    outT_d = nc.dram_tensor("outT", [RT, SHARD], dt, kind="ExternalOutput").ap()

    with tile.TileContext(nc) as tc:
        with tc.tile_pool(name="wp", bufs=1) as wp, \
             tc.tile_pool(name="sb", bufs=1) as sb, \
             tc.tile_pool(name="ps", bufs=1, space="PSUM") as ps:

            # ---- load inputs to SBUF ----
            def load(dram_ap, shape, tag):
                t = wp.tile(shape, dt, tag=tag)
                nc.sync.dma_start(t[:], dram_ap[:])
                return t

            ta_sb = load(ta_d, [H, RT * APT], "ta")
            WselfT = load(WselfT_d, [H, H], "WselfT")
            WmLT = load(WmLT_d, [H, H], "WmLT")
            WmRT = load(WmRT_d, [H, H], "WmRT")
            WtT = load(WtT_d, [H, RT * H], "WtT")
            btT = load(btT_d, [H, RT], "btT")
            bsc = load(bsc_d, [H, 1], "bsc")
            bmc = load(bmc_d, [H, 1], "bmc")
            hidc = load(hidc_d, [H, 1], "hidc")
            WlT = [load(WlT0_d, [H, 128], "WlT0"), load(WlT1_d, [H, 128], "WlT1")]
            WrT = load(WrT_d, [H, HEADS * HID], "WrT")
            Wexp = load(Wexp_d, [128, 128], "Wexp")
            fold = load(fold_d, [128, H], "fold")
            Wd0a_aug = load(Wd0a_aug_d, [H + 1, 64], "Wd0a")
            Wd0bT = load(Wd0bT_d, [H, 64], "Wd0b")
            Wd1_aug = load(Wd1_aug_d, [65, 128], "Wd1")
            Wd2T = load(Wd2T_d, [128, RT], "Wd2")
            bd2c = load(bd2c_d, [RT, 1], "bd2c")

            hT = wp.tile([H, N], dt, tag="hT")
            nc.sync.dma_start(hT[:, 1:N], ambT_d[:])
            mlp_aug = wp.tile([H + 1, SHARD], dt, tag="mlpa")
            nc.sync.dma_start(mlp_aug[0:H, :], mlp_d[:])
            nc.vector.memset(mlp_aug[H:H + 1, :], 1.0)
            # preload ACT tables (Exp/Sigmoid) off the critical softmax path
            warm = wp.tile([1, 4], dt, tag="warm")
            nc.vector.memset(warm[:], 0.0)
            warm_act = wp.tile([1, 4], dt, tag="warmact")
            nc.scalar.activation(warm_act[0:1, 0:1], warm[0:1, 0:1], AF.Exp)

            def leaky(out_ap, in_ap):
                # in_ap must be SBUF (stt can read at most one PSUM input)
                nc.vector.scalar_tensor_tensor(out=out_ap, in0=in_ap, scalar=SLOPE,
                                               in1=in_ap, op0=ALU.mult, op1=ALU.max)

            def leaky_psum(out_ap, psum_ap, scratch_ap):
                # leaky(x) = max(0.2*x, x) with x in PSUM: two DVE ops
                nc.vector.tensor_scalar_mul(scratch_ap, psum_ap, SLOPE)
                nc.vector.tensor_tensor(out_ap, scratch_ap, psum_ap, op=ALU.max)

            # ---- prologue: role-type routing + merge chain -> h1 [64,1] ----
            tsum = sb.tile([H, RT], dt, tag="tsum")
            nc.vector.reduce_sum(tsum[:], ta_sb[:].rearrange("p (t a) -> p t a", a=APT),
                                 axis=AX.X)
            tmean = sb.tile([H, RT], dt, tag="tmean")
            nc.vector.tensor_scalar_mul(tmean[:], tsum[:], 1.0 / APT)
            tmp_ps = ps.tile([H, RT], dt, tag="sp", bufs=1)
            for t in range(RT):
                nc.tensor.matmul(tmp_ps[:, t:t + 1], WtT[:, H * t:H * (t + 1)],
                                 tmean[:, t:t + 1], start=True, stop=True)
            tmpc = sb.tile([H, RT], dt, tag="tmpc")
            nc.vector.tensor_add(tmpc[:], tmp_ps[:], btT[:])
            C_ps = ps.tile([H, RT], dt, tag="sp", bufs=1)
            nc.tensor.matmul(C_ps[:], WmRT[:], tmpc[:], start=True, stop=True)
            C_sb = sb.tile([H, RT], dt, tag="C")
            nc.scalar.activation(C_sb[:], C_ps[:], AF.Identity, bias=bmc[:])

            h1_ps = ps.tile([H, 1], dt, tag="sp", bufs=1)
            nc.tensor.matmul(h1_ps[:], WselfT[:], hidc[:], start=True, stop=True)
            h1 = sb.tile([H, 1], dt, tag="h1", bufs=2)
            nc.scalar.activation(h1[:], h1_ps[:], AF.Identity, bias=bsc[:])
            for t in range(RT):
                hp = ps.tile([H, 1], dt, tag="sp", bufs=1)
                nc.tensor.matmul(hp[:], WmLT[:], h1[:], start=True, stop=True)
                u = sb.tile([H, 1], dt, tag="u", bufs=2)
                nc.scalar.activation(u[:], hp[:], AF.Identity, bias=C_sb[:, t:t + 1])
                h1n = sb.tile([H, 1], dt, tag="h1", bufs=2)
                leaky(h1n[:], u[:])
                h1 = h1n
            nc.vector.tensor_copy(hT[:, 0:1], h1[:])

            # ---- GAT row 0, two head-pair blocks ----
            h2_ps = ps.tile([H, 1], dt, tag="h2ps", bufs=1)
            for b in range(2):
                # g_r0 column for this head-pair block (attention query side)
                gr0_ps = ps.tile([128, 1], dt, tag="sp", bufs=1)
                nc.tensor.matmul(gr0_ps[:], WrT[:, 128 * b:128 * b + 128], h1[:],
                                 start=True, stop=True)
                gr0c = sb.tile([128, 1], dt, tag="gr0", bufs=2)
                nc.vector.tensor_copy(gr0c[:], gr0_ps[:])
                gl_ps = ps.tile([128, N], dt, tag="gle", bufs=2)
                for c in (0, 512):
                    nc.tensor.matmul(gl_ps[:, c:c + 512], WlT[b][:], hT[:, c:c + 512],
                                     start=True, stop=True)
                t_sb = sb.tile([128, N], dt, tag="t", bufs=2)
                for c in (0, 512):
                    u_sb = sb.tile([128, 512], dt, tag="scr", bufs=2)
                    nc.scalar.activation(u_sb[:], gl_ps[:, c:c + 512], AF.Identity,
                                         bias=gr0c[:])
                    leaky(t_sb[:, c:c + 512], u_sb[:])
                gr_ps = ps.tile([128, N], dt, tag="gr", bufs=1)
                for c in (0, 512):
                    nc.tensor.matmul(gr_ps[:, c:c + 512],
                                     WrT[:, 128 * b:128 * b + 128],
                                     hT[:, c:c + 512], start=True, stop=True)
                e_ps = ps.tile([128, N], dt, tag="gle", bufs=2)
                for c in (0, 512):
                    nc.tensor.matmul(e_ps[:, c:c + 512], Wexp[:], t_sb[:, c:c + 512],
                                     start=True, stop=True)
                # softmax over the 1024 source nodes (per head, replicated x64).
                # logits are O(5) so no max subtraction is needed in fp32.
                pexp = sb.tile([128, N], dt, tag="pexp", bufs=2)
                sa = sb.tile([128, 1], dt, tag="s", bufs=4)
                sbb = sb.tile([128, 1], dt, tag="s", bufs=4)
                nc.scalar.activation(pexp[:, 0:512], e_ps[:, 0:512], AF.Exp,
                                     bias=0.0, accum_out=sa[:])
                nc.scalar.activation(pexp[:, 512:N], e_ps[:, 512:N], AF.Exp,
                                     bias=0.0, accum_out=sbb[:])
                ssum = sb.tile([128, 1], dt, tag="s", bufs=4)
                nc.vector.tensor_add(ssum[:], sa[:], sbb[:])
                # weighted value sum over source nodes (fused mul + row-sum)
                acc = []
                for c in (0, 512):
                    scr = sb.tile([128, 512], dt, tag="scr", bufs=2)
                    a_c = sb.tile([128, 1], dt, tag="acc", bufs=4)
                    nc.vector.scalar_tensor_tensor(
                        out=scr[:], in0=pexp[:, c:c + 512], scalar=1.0,
                        in1=gr_ps[:, c:c + 512], op0=ALU.mult, op1=ALU.mult,
                        accum_out=a_c[:])
                    acc.append(a_c)
                att_u = sb.tile([128, 1], dt, tag="acc", bufs=4)
                nc.vector.tensor_add(att_u[:], acc[0][:], acc[1][:])
                rs = sb.tile([128, 1], dt, tag="s", bufs=4)
                nc.vector.reciprocal(rs[:], ssum[:])
                att_n = sb.tile([128, 1], dt, tag="acc", bufs=4)
                nc.vector.tensor_mul(att_n[:], att_u[:], rs[:])
                # fold heads: h2 += 0.25 * sum over the 2 heads in this block
                nc.tensor.matmul(h2_ps[:], fold[:], att_n[:], start=(b == 0),
                                 stop=(b == 1))

            h2 = sb.tile([H, 1], dt, tag="h2")
            nc.vector.tensor_copy(h2[:], h2_ps[:])

            # ---- final MLP on this core's 128-node shard ----
            c0_ps = ps.tile([H, 1], dt, tag="sp", bufs=1)
            nc.tensor.matmul(c0_ps[:], Wd0bT[:], h2[:], start=True, stop=True)
            c0col = sb.tile([H, 1], dt, tag="c0")
            nc.vector.tensor_copy(c0col[:], c0_ps[:])
            y0_ps = ps.tile([64, SHARD], dt, tag="sp", bufs=1)
            nc.tensor.matmul(y0_ps[:], Wd0a_aug[:], mlp_aug[:], start=True, stop=True)
            y0_aug = sb.tile([65, SHARD], dt, tag="y0")
            nc.vector.memset(y0_aug[64:65, :], 1.0)
            y0u = sb.tile([64, SHARD], dt, tag="yscr", bufs=2)
            nc.scalar.activation(y0u[:], y0_ps[:], AF.Identity, bias=c0col[:])
            leaky(y0_aug[0:64, :], y0u[:])
            y1_ps = ps.tile([128, SHARD], dt, tag="sp", bufs=1)
            nc.tensor.matmul(y1_ps[:], Wd1_aug[:], y0_aug[:], start=True, stop=True)
            y1 = sb.tile([128, SHARD], dt, tag="y1")
            y1scr = sb.tile([128, SHARD], dt, tag="yscr", bufs=2)
            leaky_psum(y1[:], y1_ps[:], y1scr[:])
            o_ps = ps.tile([RT, SHARD], dt, tag="sp", bufs=1)
            nc.tensor.matmul(o_ps[:], Wd2T[:], y1[:], start=True, stop=True)
            # sigmoid(z) = 1/(1+exp(-z)) using the already-loaded Exp table
            # (avoids a 1.3us Sigmoid ACT-table load on the critical path)
            o_e = sb.tile([RT, SHARD], dt, tag="oe")
            nc.scalar.activation(o_e[:], o_ps[:], AF.Exp, bias=bd2c[:], scale=-1.0)
            o_1p = sb.tile([RT, SHARD], dt, tag="o1p")
            nc.vector.tensor_scalar_add(o_1p[:], o_e[:], 1.0)
            o_sb = sb.tile([RT, SHARD], dt, tag="o")
            nc.vector.reciprocal(o_sb[:], o_1p[:])
            nc.sync.dma_start(outT_d[:], o_sb[:])

    nc.compile()
    return nc


def _prep_inputs(inputs):
    f32 = np.float32

    def c(a):
        return np.ascontiguousarray(a, dtype=f32)

    hidden = np.asarray(inputs["hidden"], f32)
    ambiguous = np.asarray(inputs["ambiguous"], f32)
    type_agents = np.asarray(inputs["type_agents"], f32)
    W_self = np.asarray(inputs["W_self"], f32)
    b_self = np.asarray(inputs["b_self"], f32)
    W_merge = np.asarray(inputs["W_merge"], f32)
    b_merge = np.asarray(inputs["b_merge"], f32)
    W_trans = np.asarray(inputs["W_trans"], f32)
    b_trans = np.asarray(inputs["b_trans"], f32)
    W_l = np.asarray(inputs["W_l"], f32)
    W_r = np.asarray(inputs["W_r"], f32)
    w_attn = np.asarray(inputs["w_attn"], f32)
    Wd0 = np.asarray(inputs["Wd0"], f32)
    bd0 = np.asarray(inputs["bd0"], f32)
    Wd1 = np.asarray(inputs["Wd1"], f32)
    bd1 = np.asarray(inputs["bd1"], f32)
    Wd2 = np.asarray(inputs["Wd2"], f32)
    bd2 = np.asarray(inputs["bd2"], f32)

    ambT = c(ambiguous.T)                                   # [64, 1023]
    WlT_full = c(W_l.T)                                     # [64, 256]
    Wexp = np.zeros((128, 128), f32)
    for hh in range(2):
        Wexp[hh * 64:(hh + 1) * 64, hh * 64:(hh + 1) * 64] = w_attn[:, None]
    fold = np.zeros((128, 64), f32)
    fold[np.arange(128), np.arange(128) % 64] = 0.25

    shared = {
        "ambT": ambT,
        "hidc": c(hidden.reshape(H, 1)),
        "ta": c(type_agents.reshape(RT * APT, H).T),
        "WselfT": c(W_self.T),
        "WmLT": c(W_merge[:, :H].T),
        "WmRT": c(W_merge[:, H:].T),
        "WtT": c(np.concatenate([W_trans[t].T for t in range(RT)], axis=1)),
        "btT": c(b_trans.T),
        "bsc": c(b_self.reshape(H, 1)),
        "bmc": c(b_merge.reshape(H, 1)),
        "WlT0": c(WlT_full[:, :128]),
        "WlT1": c(WlT_full[:, 128:]),
        "WrT": c(W_r.T),
        "Wexp": Wexp,
        "fold": fold,
        "Wd0a_aug": c(np.vstack([Wd0[:, :H].T, bd0[None, :]])),
        "Wd0bT": c(Wd0[:, H:].T),
        "Wd1_aug": c(np.vstack([Wd1.T, bd1[None, :]])),
        "Wd2T": c(Wd2.T),
        # negated: used as the bias of Exp(-z) inside the exp-based sigmoid
        "bd2c": c(-bd2.reshape(RT, 1)),
    }
    amb_pad = np.zeros((H, NCORES * SHARD), f32)
    amb_pad[:, :N_AMB] = ambT
    in_maps = []
    for cidx in range(NCORES):
        m = dict(shared)
        m["mlp_cols"] = c(amb_pad[:, cidx * SHARD:(cidx + 1) * SHARD])
        in_maps.append(m)
    return in_maps


def kernel(**inputs) -> np.ndarray:
    global _compiled
    if _compiled is None:
        _compiled = _build()
    nc = _compiled
    from concourse import bass_utils

    in_maps = _prep_inputs(inputs)
    res = bass_utils.run_bass_kernel_spmd(nc, in_maps, core_ids=list(range(NCORES)))
    out = np.empty((N_AMB, RT), np.float32)
    for cidx in range(NCORES):
        lo = cidx * SHARD
        hi = min(lo + SHARD, N_AMB)
        out[lo:hi, :] = res.results[cidx]["outT"][:, :hi - lo].T
    return out
